# revision 1
# baseline (speedup 1.0000x reference)
"""DimeNet edge_init (DimePredictor) Bass/Trainium2 kernel.

Strategy (8 NeuronCores, triplet/T-parallel per sharding hint):
  - T = 4M line-graph edges sharded as 500k per core (padded to 507,904 =
    128 partitions x 3968 for tiling); tables replicated per core.
  - Host-side prep is LAYOUT ONLY: build a fused table tbl48 = [rbf_env(42) |
    o(3) | pad(3)] (192B rows) and o4 = [o(3) | pad(1)] (16B rows), convert
    int64 indices to int32, pad/reshape. All FP math happens on device.
  - Per tile of 128x128 triplets: two indirect-DMA gathers per triplet column
    (fused row by src -> rbf AND R1 in one descriptor; o4 row by dst -> R2),
    then DVE/ACT compute:
        c = (R1.R2) / sqrt(|R1|^2 |R2|^2)   (== cos(atan2(|R1xR2|, R1.R2)))
        P_l(c) via Legendre recurrence, cbf = coef_l * P_l repeated 6x,
        sbf = rbf[src] * cbf
    and one sequential output DMA.
"""
import math
import numpy as np

NUM_SPHERICAL = 7
NUM_RADIAL = 6
D_OUT = NUM_SPHERICAL * NUM_RADIAL  # 42
E_ROWS = 1_000_000
T_FULL = 4_000_000
N_CORES = 8
K_TILE = 128          # triplet columns per tile
J_COLS = 3968         # columns per partition per core (31 tiles x 128)
T_CORE = T_FULL // N_CORES          # 500_000
T_PAD = 128 * J_COLS                # 507_904
FUSED_W = 48          # 42 rbf + 3 o + 3 pad
O4_W = 4              # 3 o + 1 pad

_CACHE = {}


def build_program(j_cols=J_COLS, k_tile=K_TILE, n_cores=N_CORES, repeat=1):
    import concourse.bacc as bacc
    import concourse.bass as bass
    import concourse.tile as tile
    import concourse.mybir as mybir

    f32 = mybir.dt.float32
    nc = bacc.Bacc("TRN2", target_bir_lowering=False, debug=False, num_devices=n_cores)
    tbl = nc.dram_tensor("tbl48", [E_ROWS, FUSED_W], f32, kind="ExternalInput").ap()
    o4 = nc.dram_tensor("o4", [E_ROWS, O4_W], f32, kind="ExternalInput").ap()
    srci = nc.dram_tensor("srci", [128, j_cols], mybir.dt.int32, kind="ExternalInput").ap()
    dsti = nc.dram_tensor("dsti", [128, j_cols], mybir.dt.int32, kind="ExternalInput").ap()
    out = nc.dram_tensor("out", [128, j_cols * D_OUT], f32, kind="ExternalOutput").ap()

    # Legendre recurrence, scaled form:
    #   P_l = a_l * c * P_{l-1} - b_l * P_{l-2},  a_l=(2l-1)/l, b_l=(l-1)/l
    # Use G_l := P_l / g_l with g_l = a_l * g_{l-1} so that
    #   G_l = c * G_{l-1} - b2_l * G_{l-2},  b2_l = b_l * g_{l-2} / (a_l * g_{l-1})
    # Final scale per l: q_l = coef_l * g_l  (coef_l = sqrt((2l+1)/4pi)).
    g = [1.0, 1.0]
    for l in range(2, NUM_SPHERICAL):
        a_l = (2 * l - 1) / l
        g.append(a_l * g[-1])
    b2 = {}
    for l in range(2, NUM_SPHERICAL):
        a_l = (2 * l - 1) / l
        b_l = (l - 1) / l
        b2[l] = b_l * g[l - 2] / (a_l * g[l - 1])
    coef = [float(np.sqrt((2 * l + 1) / (4.0 * np.pi)).astype(np.float32))
            for l in range(NUM_SPHERICAL)]
    qscale = [coef[l] * g[l] for l in range(NUM_SPHERICAL)]

    K = k_tile
    n_tiles = j_cols // K
    assert n_tiles * K == j_cols

    with tile.TileContext(nc) as tc:
        with tc.tile_pool(name="idxp", bufs=1) as idxp, \
             tc.tile_pool(name="ftp", bufs=2) as ftp, \
             tc.tile_pool(name="odp", bufs=2) as odp, \
             tc.tile_pool(name="otp", bufs=2) as otp, \
             tc.tile_pool(name="tmp", bufs=2) as tmp:
            src_t = idxp.tile([128, j_cols], mybir.dt.int32)
            dst_t = idxp.tile([128, j_cols], mybir.dt.int32)
            nc.sync.dma_start(src_t[:], srci[:])
            nc.sync.dma_start(dst_t[:], dsti[:])

            for _rep in range(repeat):
                for it in range(n_tiles):
                    c0 = it * K
                    ft = ftp.tile([128, K * FUSED_W], f32)
                    od = odp.tile([128, K * O4_W], f32)
                    for j in range(K):
                        nc.gpsimd.indirect_dma_start(
                            out=ft[:, j * FUSED_W:(j + 1) * FUSED_W],
                            out_offset=None,
                            in_=tbl[:],
                            in_offset=bass.IndirectOffsetOnAxis(
                                ap=src_t[:, c0 + j:c0 + j + 1], axis=0),
                        )
                        nc.gpsimd.indirect_dma_start(
                            out=od[:, j * O4_W:(j + 1) * O4_W],
                            out_offset=None,
                            in_=o4[:],
                            in_offset=bass.IndirectOffsetOnAxis(
                                ap=dst_t[:, c0 + j:c0 + j + 1], axis=0),
                        )

                    ft3 = ft[:].rearrange("p (k f) -> p k f", f=FUSED_W)
                    od3 = od[:].rearrange("p (k f) -> p k f", f=O4_W)
                    R1 = ft3[:, :, D_OUT:D_OUT + 3]
                    R2 = od3[:, :, 0:3]

                    m = tmp.tile([128, K * 3], f32, tag="m")
                    m3 = m[:].rearrange("p (k f) -> p k f", f=3)
                    sc = tmp.tile([128, K * 8], f32, tag="sc")  # scalar lanes
                    dot = sc[:, 0 * K:1 * K]
                    n1 = sc[:, 1 * K:2 * K]
                    n2 = sc[:, 2 * K:3 * K]
                    cc = sc[:, 3 * K:4 * K]
                    w0 = sc[:, 4 * K:5 * K]
                    g_prev = sc[:, 5 * K:6 * K]   # G_{l-1}
                    g_prev2 = sc[:, 6 * K:7 * K]  # G_{l-2}
                    g_cur = sc[:, 7 * K:8 * K]

                    mul = mybir.AluOpType.mult
                    add = mybir.AluOpType.add
                    sub = mybir.AluOpType.subtract

                    def lanes(ap_flat):
                        return ap_flat.rearrange("p (k one) -> p k one", one=1)

                    # dot = sum R1*R2 ; n1 = |R1|^2 ; n2 = |R2|^2
                    nc.vector.tensor_tensor(out=m3[:], in0=R1, in1=R2, op=mul)
                    nc.vector.tensor_tensor(out=dot, in0=m[:, 0::3], in1=m[:, 1::3], op=add)
                    nc.vector.tensor_tensor(out=dot, in0=dot, in1=m[:, 2::3], op=add)
                    nc.vector.tensor_tensor(out=m3[:], in0=R1, in1=R1, op=mul)
                    nc.vector.tensor_tensor(out=n1, in0=m[:, 0::3], in1=m[:, 1::3], op=add)
                    nc.vector.tensor_tensor(out=n1, in0=n1, in1=m[:, 2::3], op=add)
                    nc.vector.tensor_tensor(out=m3[:], in0=R2, in1=R2, op=mul)
                    nc.vector.tensor_tensor(out=n2, in0=m[:, 0::3], in1=m[:, 1::3], op=add)
                    nc.vector.tensor_tensor(out=n2, in0=n2, in1=m[:, 2::3], op=add)
                    # c = dot * rsqrt(n1*n2), seed via ACT sqrt + DVE reciprocal,
                    # then one Newton step on rsqrt: y <- y*(1.5 - 0.5*p*y^2)
                    p_ = sc[:, 4 * K:5 * K]
                    nc.vector.tensor_tensor(out=p_, in0=n1, in1=n2, op=mul)
                    r_ = sc[:, 7 * K:8 * K]  # g_cur lane free until Legendre
                    nc.scalar.sqrt(out=r_, in_=p_)
                    nc.vector.reciprocal(out=r_, in_=r_)
                    t_ = sc[:, 5 * K:6 * K]  # g_prev lane free until Legendre
                    nc.vector.tensor_tensor(out=t_, in0=r_, in1=r_, op=mul)
                    nc.vector.tensor_tensor(out=t_, in0=t_, in1=p_, op=mul)
                    nc.vector.tensor_scalar(out=t_, in0=t_, scalar1=-0.5, scalar2=1.5,
                                            op0=mul, op1=add)
                    nc.vector.tensor_tensor(out=r_, in0=r_, in1=t_, op=mul)
                    nc.vector.tensor_tensor(out=cc, in0=dot, in1=r_, op=mul)

                    ot = otp.tile([128, K * D_OUT], f32)
                    ot3 = ot[:].rearrange("p (k f) -> p k f", f=D_OUT)

                    def emit_l(l, G_ap):
                        # out[:, :, 6l:6l+6] = (qscale_l * G_l) * rbf[:, :, 6l:6l+6]
                        nc.vector.tensor_scalar(
                            out=w0, in0=G_ap, scalar1=float(qscale[l]), scalar2=None,
                            op0=mul)
                        nc.vector.tensor_tensor(
                            out=ot3[:, :, l * NUM_RADIAL:(l + 1) * NUM_RADIAL],
                            in0=ft3[:, :, l * NUM_RADIAL:(l + 1) * NUM_RADIAL],
                            in1=lanes(w0).to_broadcast([128, K, NUM_RADIAL]),
                            op=mul)

                    # l = 0: G_0 = 1 -> out = coef0 * rbf
                    nc.vector.tensor_scalar(
                        out=ot3[:, :, 0:NUM_RADIAL],
                        in0=ft3[:, :, 0:NUM_RADIAL],
                        scalar1=float(qscale[0]), scalar2=None, op0=mul)
                    # l = 1: G_1 = c
                    emit_l(1, cc)
                    # G_0 = 1, G_1 = c
                    nc.vector.memset(g_prev2, 1.0)
                    nc.vector.tensor_copy(out=g_prev, in_=cc)
                    for l in range(2, NUM_SPHERICAL):
                        # G_l = c*G_{l-1} - b2_l*G_{l-2}
                        nc.vector.tensor_tensor(out=g_cur, in0=cc, in1=g_prev, op=mul)
                        nc.vector.tensor_scalar(
                            out=g_prev2, in0=g_prev2, scalar1=float(b2[l]),
                            scalar2=None, op0=mul)
                        nc.vector.tensor_tensor(out=g_cur, in0=g_cur, in1=g_prev2, op=sub)
                        emit_l(l, g_cur)
                        # rotate: G_{l-2} <- G_{l-1}, G_{l-1} <- G_l
                        nc.vector.tensor_copy(out=g_prev2, in_=g_prev)
                        nc.vector.tensor_copy(out=g_prev, in_=g_cur)

                    nc.sync.dma_start(
                        out[:, c0 * D_OUT:(c0 + K) * D_OUT], ot[:])

    nc.compile()
    return nc


def _get_runner(nc, n_cores):
    """Build a jitted SPMD executor for the compiled Bass program."""
    import jax
    import jax.numpy as jnp
    from jax.sharding import Mesh, PartitionSpec, NamedSharding
    from jax.experimental.shard_map import shard_map
    import concourse.mybir as mybir
    from concourse.bass2jax import _bass_exec_p, install_neuronx_cc_hook, partition_id_tensor

    install_neuronx_cc_hook()
    partition_name = nc.partition_id_tensor.name if nc.partition_id_tensor else None
    in_names, out_names, out_avals = [], [], []
    for alloc in nc.m.functions[0].allocations:
        if not isinstance(alloc, mybir.MemoryLocationSet):
            continue
        name = alloc.memorylocations[0].name
        if alloc.kind == "ExternalInput":
            if name != partition_name:
                in_names.append(name)
        elif alloc.kind == "ExternalOutput":
            out_names.append(name)
            out_avals.append(jax.core.ShapedArray(
                tuple(alloc.tensor_shape), mybir.dt.np(alloc.dtype)))
    n_params = len(in_names)
    n_outs = len(out_avals)
    all_in_names = in_names + out_names
    if partition_name is not None:
        all_in_names = all_in_names + [partition_name]
    donate = tuple(range(n_params, n_params + n_outs))

    def _body(*args):
        operands = list(args)
        if partition_name is not None:
            operands.append(partition_id_tensor())
        outs = _bass_exec_p.bind(
            *operands,
            out_avals=tuple(out_avals),
            in_names=tuple(all_in_names),
            out_names=tuple(out_names),
            lowering_input_output_aliases=(),
            sim_require_finite=True,
            sim_require_nnan=True,
            nc=nc,
        )
        return tuple(outs)

    try:
        devices = jax.devices("axon")[:n_cores]
    except RuntimeError:
        devices = jax.devices()[:n_cores]
    mesh = Mesh(np.asarray(devices), ("core",))
    sharded = jax.jit(
        shard_map(_body, mesh=mesh,
                  in_specs=(PartitionSpec("core"),) * (n_params + n_outs),
                  out_specs=(PartitionSpec("core"),) * n_outs,
                  check_rep=False),
        donate_argnums=donate,
        keep_unused=True,
    )
    shard0 = NamedSharding(mesh, PartitionSpec("core"))

    def make_zeros():
        return [
            jax.jit(
                lambda shape=av.shape, dt=av.dtype: jnp.zeros(
                    (n_cores * shape[0],) + tuple(shape[1:]), dt),
                out_shardings=shard0,
            )()
            for av in out_avals
        ]

    return sharded, in_names, out_names, out_avals, shard0, make_zeros


def prep_inputs(o, rbf_env, src_idx, dst_idx):
    """Host-side layout-only prep. Returns concatenated per-core input arrays."""
    o = np.asarray(o, dtype=np.float32)
    rbf = np.asarray(rbf_env, dtype=np.float32)
    src = np.asarray(src_idx).astype(np.int32)
    dst = np.asarray(dst_idx).astype(np.int32)
    assert o.shape == (E_ROWS, 3) and rbf.shape == (E_ROWS, D_OUT)
    assert src.shape == (T_FULL,) and dst.shape == (T_FULL,)

    tbl48 = np.zeros((E_ROWS, FUSED_W), dtype=np.float32)
    tbl48[:, :D_OUT] = rbf
    tbl48[:, D_OUT:D_OUT + 3] = o
    o4 = np.zeros((E_ROWS, O4_W), dtype=np.float32)
    o4[:, :3] = o

    src_pads, dst_pads = [], []
    for c in range(N_CORES):
        s = src[c * T_CORE:(c + 1) * T_CORE]
        d = dst[c * T_CORE:(c + 1) * T_CORE]
        sp = np.zeros(T_PAD, dtype=np.int32)
        dp = np.zeros(T_PAD, dtype=np.int32)
        sp[:T_CORE] = s
        dp[:T_CORE] = d
        src_pads.append(sp.reshape(128, J_COLS))
        dst_pads.append(dp.reshape(128, J_COLS))

    concat = {
        "tbl48": np.concatenate([tbl48] * N_CORES, axis=0),
        "o4": np.concatenate([o4] * N_CORES, axis=0),
        "srci": np.concatenate(src_pads, axis=0),
        "dsti": np.concatenate(dst_pads, axis=0),
    }
    return concat


def assemble_output(out_concat):
    """out_concat: [N_CORES*128, J_COLS*42] -> [T_FULL, 42]."""
    parts = []
    for c in range(N_CORES):
        blk = out_concat[c * 128:(c + 1) * 128]          # [128, J*42]
        rows = blk.reshape(128 * J_COLS, D_OUT)           # triplet (p*J+j) -> row
        parts.append(rows[:T_CORE])
    return np.concatenate(parts, axis=0)


def kernel(o, rbf_env, src_idx, dst_idx):
    import jax

    if "prog" not in _CACHE:
        _CACHE["prog"] = build_program()
        _CACHE["runner"] = _get_runner(_CACHE["prog"], N_CORES)
    sharded, in_names, out_names, out_avals, shard0, make_zeros = _CACHE["runner"]

    concat = prep_inputs(o, rbf_env, src_idx, dst_idx)
    dev_in = [jax.device_put(concat[name], shard0) for name in in_names]
    outs = sharded(*dev_in, *make_zeros())
    jax.block_until_ready(outs)
    out_concat = np.asarray(outs[out_names.index("out")])
    return assemble_output(out_concat)

